# revision 4
# baseline (speedup 1.0000x reference)
"""Trainium2 Bass kernel v7 for nn_LlamaAttention (B=1, S=2048, D=4096, H=32, KVH=8, HD=128).

v7 over v5:
- Batched softmax denominators: each head's ones-matmul uses a [128,4]
  selector stationary that writes its own row of one per-chunk [4,512] PSUM
  tile, so the slow DVE reciprocal (512 elems/lane regardless of partition
  count) runs once per chunk instead of once per head: 13us total vs 53us,
  removing the DVE bottleneck at the start of phase 2.
- Q^T/K^T/V/attention-output storage split into per-chunk tiles: the
  dependency tracker is tile-granular, so with monolithic tensors phase 2
  waited on the last RoPE write and phase 3 on the last normalization write.
- prb staging pool deepened to 4 buffers to decouple consecutive tails.
"""

import math

import numpy as np

S = 2048
D = 4096
H = 32
KVH = 8
HD = 128
ROT = 64
HALF = 32
THETA = 10000.0
NCORES = 8
QH = H // NCORES
P = 128
CH = 512
NCH = S // CH  # 4
DT = D // P  # 32
KT = S // P  # 16
GK = 4  # kt tiles per xt/weight DMA group
NG = DT // GK  # 8

_CACHE = {}


def _build_nc():
    from collections import deque

    import concourse.mybir as mybir
    from concourse import bacc
    from concourse.bass import ds
    from concourse.masks import make_identity
    from concourse.tile import TileContext

    f32 = mybir.dt.float32
    f32r = mybir.dt.float32r
    bf16 = mybir.dt.bfloat16
    EXP = mybir.ActivationFunctionType.Exp

    nc = bacc.Bacc()

    xp = nc.dram_tensor("xp", [P, NCH * DT * CH], bf16, kind="ExternalInput")
    wq = nc.dram_tensor("wq", [P, DT * QH * HD], bf16, kind="ExternalInput")
    wk = nc.dram_tensor("wk", [P, DT * HD], bf16, kind="ExternalInput")
    wv = nc.dram_tensor("wv", [P, DT * HD], bf16, kind="ExternalInput")
    wo = nc.dram_tensor("wo", [P, QH * D], bf16, kind="ExternalInput")
    costab = nc.dram_tensor("costab", [ROT, S], bf16, kind="ExternalInput")
    sintab = nc.dram_tensor("sintab", [ROT, S], bf16, kind="ExternalInput")
    maskt = nc.dram_tensor("maskt", [P, P], bf16, kind="ExternalInput")
    seltab = nc.dram_tensor("seltab", [P, QH * QH], bf16, kind="ExternalInput")
    out = nc.dram_tensor("out", [S, D], bf16, kind="ExternalOutput")

    with TileContext(nc) as tc:
        with tc.tile_pool(name="pconst", bufs=1) as pconst, \
             tc.tile_pool(name="pact", bufs=1) as pact:
            wq_sb = pconst.tile([P, DT, QH * HD], bf16)
            wk_sb = pconst.tile([P, DT, HD], bf16)
            wv_sb = pconst.tile([P, DT, HD], bf16)

            # First wq slice (2 kt) on the sync ring; the rest prefetched
            # group-by-group inside chunk 0. wk/wv whole on the scalar ring
            # so the first kt's K/V weights land almost immediately.
            nc.sync.dma_start(wq_sb[:, ds(0, 2)], wq[:, ds(0, 2 * QH * HD)])
            for g in range(2):
                nc.scalar.dma_start(wk_sb[:, ds(g * 16, 16)],
                                    wk[:, ds(g * 16 * HD, 16 * HD)])
                nc.scalar.dma_start(wv_sb[:, ds(g * 16, 16)],
                                    wv[:, ds(g * 16 * HD, 16 * HD)])

            costab_sb = pconst.tile([ROT, S], bf16)
            nc.scalar.dma_start(costab_sb[:], costab[:])
            sintab_sb = pconst.tile([ROT, S], bf16)
            nc.scalar.dma_start(sintab_sb[:], sintab[:])
            maskt_sb = pconst.tile([P, P], bf16)
            nc.scalar.dma_start(maskt_sb[:], maskt[:])
            sel_sb = pconst.tile([P, QH, QH], bf16)
            nc.scalar.dma_start(sel_sb[:], seltab[:])
            ident = pconst.tile([P, P], bf16)
            make_identity(nc, ident[:])
            ones_sb = pconst.tile([P, 1], bf16)
            nc.gpsimd.memset(ones_sb[:], 1.0)
            # Warm up the exp activation table while startup DMAs stream.
            warm = pconst.tile([P, 1], bf16)
            nc.scalar.activation(warm[:], ones_sb[:], EXP)

            wo_sb = pconst.tile([P, QH, D], bf16)
            for h in range(QH):
                nc.scalar.dma_start(wo_sb[:, h], wo[:, ds(h * D, D)])

            # Per-chunk tiles: the dependency tracker is tile-granular, so
            # phase 2/3 consumers must not chain to later chunks' writes.
            qtc = [pact.tile([P, QH, CH], bf16, name=f"qtc{c}")
                   for c in range(NCH)]
            kts = [pact.tile([P, CH], bf16, name=f"kts{c}")
                   for c in range(NCH)]
            vs = [pact.tile([P, 4, HD], bf16, name=f"vs{c}")
                  for c in range(NCH)]
            aoutc = [pact.tile([P, QH, CH], bf16, name=f"aoutc{c}")
                     for c in range(NCH)]

            # Warm the PE clock (HAM un-throttles after ~3.4us of activity)
            # with dummy matmuls on the identity while startup DMAs stream.
            with tc.tile_pool(name="pwm", bufs=1, space="PSUM") as pwm:
                wps = pwm.tile([P, P], f32, tag="wps", name="wps")
                for _ in range(30):
                    nc.tensor.matmul(wps[:], ident[:], ident[:],
                                     start=True, stop=True)

            # ---------------- Phase 1: QKV projections ----------------
            with tc.tile_pool(name="pxt", bufs=6) as pxt, \
                 tc.tile_pool(name="prt", bufs=3) as prt, \
                 tc.tile_pool(name="pvt", bufs=2) as pvt, \
                 tc.tile_pool(name="ps1", bufs=1, space="PSUM") as ps1, \
                 tc.tile_pool(name="pst", bufs=2, space="PSUM") as pst_pool:

                def rope_tail(dst, sq):
                    rt = prt.tile([ROT, CH], bf16, tag="rt", name="rt")
                    nc.gpsimd.dma_start(rt[0:HALF], dst[HALF:ROT])
                    nc.gpsimd.dma_start(rt[HALF:ROT], dst[0:HALF])
                    nc.vector.tensor_mul(dst[0:ROT], dst[0:ROT], costab_sb[:, sq])
                    nc.vector.tensor_mul(rt[:], rt[:], sintab_sb[:, sq])
                    nc.vector.tensor_add(dst[0:ROT], dst[0:ROT], rt[:])

                # chunk 0 starts with tiny DMA groups for a faster ramp
                GROUPS0 = [2, 2, 2, 2] + [4] * 6
                for c in range(NCH):
                    sq = ds(c * CH, CH)
                    groups = GROUPS0 if c == 0 else [GK] * NG
                    pq = [ps1.tile([P, CH], f32, tag=f"pq{i}", name=f"pq{i}")
                          for i in range(QH)]
                    pk = ps1.tile([P, CH], f32, tag="pk")
                    pv = ps1.tile([P, CH], f32, tag="pv")
                    kt0 = 0
                    for g, gsz in enumerate(groups):
                        xt = pxt.tile([P, GK, CH], bf16, tag="xt", name="xt")
                        nc.sync.dma_start(
                            xt[:, 0:gsz],
                            xp[:, ds((c * DT + kt0) * CH, gsz * CH)])
                        if c == 0 and g < len(groups) - 1:
                            nkt0, nsz = kt0 + gsz, groups[g + 1]
                            nc.sync.dma_start(
                                wq_sb[:, ds(nkt0, nsz)],
                                wq[:, ds(nkt0 * QH * HD, nsz * QH * HD)])
                        for kk in range(gsz):
                            kt = kt0 + kk
                            xr = xt[:, kk]
                            st = dict(start=(kt == 0), stop=(kt == DT - 1))
                            for h in range(QH):
                                nc.tensor.matmul(
                                    pq[h][:], wq_sb[:, kt, ds(h * HD, HD)],
                                    xr, **st)
                            nc.tensor.matmul(pk[:], wk_sb[:, kt], xr, **st)
                            nc.tensor.matmul(pv[:], wv_sb[:, kt], xr, **st)
                        kt0 += gsz
                    # Copies-first drain: all six accumulators evacuate on
                    # alternating engines so their banks free ASAP, then the
                    # PE transposes and RoPE arithmetic follow.
                    vt = pvt.tile([P, CH], bf16, tag="vt", name="vt")
                    nc.vector.tensor_copy(vt[:], pv[:])
                    nc.scalar.copy(qtc[c][:, 0], pq[0][:])
                    nc.vector.tensor_copy(qtc[c][:, 1], pq[1][:])
                    nc.scalar.copy(qtc[c][:, 2], pq[2][:])
                    nc.vector.tensor_copy(qtc[c][:, 3], pq[3][:])
                    nc.scalar.copy(kts[c][:], pk[:])
                    for j in range(4):
                        pt = pst_pool.tile([P, P], bf16, tag="pst", name="pst")
                        nc.tensor.transpose(pt[:], vt[:, ds(j * P, P)], ident[:])
                        nc.vector.tensor_copy(vs[c][:, j], pt[:])
                    for h in range(QH):
                        rope_tail(qtc[c][:, h], sq)
                    rope_tail(kts[c][:], sq)

            # ---------------- Phase 2: causal attention ----------------
            # One flat pipeline over tasks (c, h, pair p); chunk-outer so the
            # first tasks only need chunk 0 of K/V/Q. The whole normalization
            # tail (reciprocal-row staging, broadcast, multiply) runs on
            # GpSimd: it is off both the PE and the DVE FIFO, and the
            # normalized outputs are not needed until phase 3.
            with tc.tile_pool(name="pes", bufs=3) as pes, \
                 tc.tile_pool(name="prb", bufs=2) as prb, \
                 tc.tile_pool(name="pob", bufs=2) as pob:
                tasks = [(c, h, p)
                         for c in range(NCH)
                         for h in range(QH)
                         for p in range(2 * (c + 1))]

                def offset(c, t):
                    j = t - 4 * c
                    return 128 * j if j >= 0 else 0

                with tc.tile_pool(name="pss", bufs=2, space="PSUM") as pss_pool, \
                     tc.tile_pool(name="psd", bufs=2, space="PSUM") as psd_pool, \
                     tc.tile_pool(name="pso", bufs=2, space="PSUM") as pso_pool:

                    def scores(c, h, p):
                        qr = qtc[c][:, h]
                        ps = pss_pool.tile([P, 2 * CH], f32, tag="pss",
                                           name="pss")
                        for half in range(2):
                            t = 2 * p + half
                            off = offset(c, t)
                            nc.tensor.matmul(
                                ps[:, ds(half * CH + off, CH - off)],
                                kts[t // 4][:, ds((t % 4) * P, P)],
                                qr[:, ds(off, CH - off)],
                                start=True, stop=True)
                        return ps

                    def consume(c, h, p, ps, psd, pso):
                        ntile = 4 * (c + 1)
                        es = pes.tile([P, 2 * CH], bf16, tag="es", name="es")
                        nc.scalar.activation(es[:], ps[:], EXP)
                        for half in range(2):
                            t = 2 * p + half
                            off = offset(c, t)
                            if t - 4 * c >= 0:
                                nc.vector.tensor_mul(
                                    es[:, ds(half * CH + off, P)],
                                    es[:, ds(half * CH + off, P)],
                                    maskt_sb[:])
                            eh = es[:, ds(half * CH + off, CH - off)]
                            first = (t == 0)
                            last = (t == ntile - 1)
                            # psd is per-chunk [QH, CH]: selector stationary
                            # writes this head's row; one accumulation group
                            # spans all four heads of the chunk.
                            nc.tensor.matmul(psd[:, ds(off, CH - off)],
                                             sel_sb[:, h], eh,
                                             start=(first and h == 0),
                                             stop=(last and h == QH - 1))
                            nc.tensor.matmul(pso[:, ds(off, CH - off)],
                                             vs[t // 4][:, t % 4], eh,
                                             start=first, stop=last)

                    def evac(pso):
                        psoc = prb.tile([P, CH], f32, tag="psoc", name="psoc",
                                        bufs=8)
                        nc.vector.tensor_copy(psoc[:], pso[:])
                        return psoc

                    def chunk_tail(c, psd, psocs):
                        rcp4 = prb.tile([QH, CH], f32, tag="rcp4",
                                        name="rcp4", bufs=2)
                        nc.vector.reciprocal(rcp4[:], psd[:])
                        for h in range(QH):
                            rcp1 = prb.tile([1, CH], f32, tag="rcp1",
                                            name="rcp1", bufs=4)
                            nc.gpsimd.dma_start(rcp1[:], rcp4[h:h + 1, :])
                            rb = prb.tile([P, CH], f32, tag="rb", name="rb",
                                          bufs=4)
                            nc.gpsimd.partition_broadcast(rb[:], rcp1[:])
                            nc.gpsimd.tensor_mul(aoutc[c][:, h], psocs[h][:],
                                                 rb[:])

                    LA = 2
                    sq_buf = deque()
                    next_sc = 0
                    tails = deque()  # (emit_at, c, psd, psocs)
                    psd_cur = None
                    pso_cur = None
                    psocs_cur = []
                    for i, (c, h, p) in enumerate(tasks):
                        while next_sc < len(tasks) and next_sc <= i + LA - 1:
                            tsk = tasks[next_sc]
                            sq_buf.append(scores(*tsk))
                            next_sc += 1
                        ps = sq_buf.popleft()
                        if h == 0 and p == 0:
                            psd_cur = psd_pool.tile([QH, CH], f32, tag="psd",
                                                    name="psd")
                            psocs_cur = []
                        if p == 0:
                            pso_cur = pso_pool.tile([P, CH], f32, tag="pso",
                                                    name="pso")
                        consume(c, h, p, ps, psd_cur, pso_cur)
                        if p == 2 * (c + 1) - 1:
                            psocs_cur.append(evac(pso_cur))
                            if h == QH - 1:
                                tails.append((i + 2, c, psd_cur, psocs_cur))
                        while tails and tails[0][0] <= i:
                            _, tc_, tpsd, tpsocs = tails.popleft()
                            chunk_tail(tc_, tpsd, tpsocs)
                    while tails:
                        _, tc_, tpsd, tpsocs = tails.popleft()
                        chunk_tail(tc_, tpsd, tpsocs)

                # ---------------- Phase 3: output projection (partial) -----
                with tc.tile_pool(name="pw3", bufs=2, space="PSUM") as pw3:
                    for i in range(KT):
                        ob = pob.tile([P, D // CH, CH], bf16, tag="ob",
                                      name="ob")
                        for j in range(D // CH):
                            pw = pw3.tile([P, CH], f32, tag="pw", name="pw")
                            for h in range(QH):
                                nc.tensor.matmul(
                                    pw[:],
                                    aoutc[i // 4][:, h, ds((i % 4) * P, P)],
                                    wo_sb[:, h, ds(j * CH, CH)],
                                    start=(h == 0), stop=(h == QH - 1))
                            if j % 2 == 0:
                                nc.vector.tensor_copy(ob[:, j], pw[:])
                            else:
                                nc.scalar.copy(ob[:, j], pw[:])
                        nc.sync.dma_start(out[ds(i * P, P), :], ob[:])

    nc.finalize()
    return nc


def _get_nc():
    if "nc" not in _CACHE:
        _CACHE["nc"] = _build_nc()
    return _CACHE["nc"]


def _host_prep(hidden_states, attention_mask, position_ids, Wq, Wk, Wv, Wo):
    import ml_dtypes
    bf = ml_dtypes.bfloat16

    x = np.asarray(hidden_states, dtype=np.float32).reshape(S, D)
    Wq = np.asarray(Wq, dtype=np.float32)
    Wk = np.asarray(Wk, dtype=np.float32)
    Wv = np.asarray(Wv, dtype=np.float32)
    Wo = np.asarray(Wo, dtype=np.float32)
    pos = np.asarray(position_ids).reshape(S).astype(np.float32)

    xT = x.T
    xprep = np.ascontiguousarray(
        xT.reshape(DT, P, NCH, CH).transpose(1, 2, 0, 3)
    ).reshape(P, NCH * DT * CH).astype(bf)

    freqs = (1.0 / THETA ** (np.arange(0, HD, 2, dtype=np.float32) / HD)
             ).astype(np.float32)
    ang = pos[:, None] * freqs[None, :]
    costab = np.ascontiguousarray(np.cos(ang).T).astype(bf)
    sint = np.sin(ang).T
    sintab = np.ascontiguousarray(
        np.concatenate([-sint[:HALF], sint[HALF:]], axis=0)).astype(bf)

    # Triangle mask for the 128-wide diagonal strip: keep iff u >= k'.
    kv = np.arange(P)
    maskt = (np.arange(P)[None, :] >= kv[:, None]).astype(np.float32)
    maskt = maskt.astype(bf)

    # selector stationaries: sel_h = [P, QH] with column h all-ones
    seltab = np.zeros((P, QH, QH), np.float32)
    for h in range(QH):
        seltab[:, h, h] = 1.0
    seltab = seltab.reshape(P, QH * QH).astype(bf)


    scale = np.float32(1.0 / math.sqrt(HD))

    def wprep(w, ncols):
        return np.ascontiguousarray(
            w.reshape(DT, P, ncols).transpose(1, 0, 2)
        ).reshape(P, DT * ncols).astype(bf)

    in_maps = []
    for c in range(NCORES):
        wq_c = Wq[:, c * QH * HD:(c + 1) * QH * HD] * scale
        wk_c = Wk[:, c * HD:(c + 1) * HD]
        wv_c = Wv[:, c * HD:(c + 1) * HD]
        wo_c = np.ascontiguousarray(
            Wo[c * QH * HD:(c + 1) * QH * HD, :].reshape(QH, P, D)
            .transpose(1, 0, 2)).reshape(P, QH * D).astype(bf)
        in_maps.append({
            "xp": xprep,
            "wq": wprep(wq_c, QH * HD),
            "wk": wprep(wk_c, HD),
            "wv": wprep(wv_c, HD),
            "wo": wo_c,
            "costab": costab,
            "sintab": sintab,
            "maskt": maskt,
            "seltab": seltab,
        })
    return in_maps


def _run(inputs, trace=False):
    from concourse.bass_utils import run_bass_kernel_spmd

    if trace:
        try:
            import antenv.axon_hooks  # noqa: F401
        except ImportError:
            import sys
            import types
            try:
                import trn_agent_boot.trn_boot as _tb
                _hook = _tb._ntff_profile_via_ctypes("/opt/axon/libaxon_pjrt.so")
                _m = types.ModuleType("antenv.axon_hooks")
                _m.get_axon_ntff_profile_hook = lambda: _hook
                _m.set_axon_ntff_profile_hook = lambda h: None
                sys.modules["antenv.axon_hooks"] = _m
            except Exception:
                trace = False

    nc = _get_nc()
    in_maps = _host_prep(**inputs)
    res = run_bass_kernel_spmd(nc, in_maps, core_ids=list(range(NCORES)),
                               trace=trace)
    acc = np.zeros((S, D), np.float32)
    for c in range(NCORES):
        acc += res.results[c]["out"].astype(np.float32)
    return np.ascontiguousarray(acc[None]), res


def kernel(hidden_states, attention_mask, position_ids, Wq, Wk, Wv, Wo):
    out, _ = _run(dict(
        hidden_states=hidden_states, attention_mask=attention_mask,
        position_ids=position_ids, Wq=Wq, Wk=Wk, Wv=Wv, Wo=Wo))
    return out
